# revision 6
# baseline (speedup 1.0000x reference)
"""Trainium2 Bass kernel for nn_BCNLayer (locally-connected 7x7 lattice layer + sigmoid).

Math: y[i,j,b] = sigmoid( sum_{dy,dx in [-3,3]} w[dy+3,dx+3][(i-dy)*W + (j-dx)]
                          * x[(i-dy)*W + (j-dx), b] )   (zero outside lattice)

Strategy:
  - 8-way shard over lattice rows (H=128 -> 16 dest rows/core, 22 source rows
    with 3-row halos, zero-padded at the edges).
  - For one dest row i and source-row offset d (7 of them), the contribution is
    a banded 128x128 matrix (band +-3 over lattice columns) applied to the
    source row's [128 cols x B batch] slab: nc.tensor.matmul(psum, lhsT=Wband,
    rhs=xrow) accumulated over the 7 source rows.  Banded matrices are prebuilt
    on the host and DMA'd in.
  - PE is the bottleneck (224 matmuls x 512 rows at 2.4GHz ~= 48.4us/core), so
    everything is scheduled around keeping it 100% busy from the earliest start:
      * tile order is t-outer / batch-chunk-inner so each wb tile serves two
        consecutive psum tiles (halves weight-stream bandwidth demand).
      * critical-first loads: tile 0's operands split across all three DMA
        paths (SP HWDGE, ACT HWDGE, Pool SWDGE) with nothing else competing.
      * bulk loads are paced: x row-groups are interleaved into the compute
        stream (engine FIFOs release them just-in-time), wb tiles gated on
        matmul progress via explicit deps, so in-flight DMA never floods the
        HBM path that the next-needed transfer is on.
      * spin matmuls keep the PE busy through the load phase so the HAM clock
        gate is warm (2.4GHz) when real matmuls start.
      * outputs are bf16 (host upcasts; bf16 keeps fp32's exponent range so
        tiny sigmoid outputs stay accurate) on the ACT/SP HWDGE rings.
"""

import os

import numpy as np

H = 128
W = 128
HW = H * W
B = 1024
NCORES = 8
T = H // NCORES  # dest rows per core = 16
S = T + 6        # source rows per core (halo 3 each side) = 22
BC = 512         # batch chunk (psum bank = 512 fp32)
NB = B // BC     # chunks = 2
NSPIN = 30       # PE warm-up matmuls during the load phase

_cache: dict = {}

# filled by the last kernel() call when KERNEL_TRACE=1
last_exec_time_ns = None
last_results = None


def _build_program():
    from contextlib import ExitStack

    import concourse.bacc as bacc
    import concourse.mybir as mybir
    import concourse.tile as tile
    from concourse.tile_rust import add_dep_helper

    nc = bacc.Bacc(
        "TRN2", target_bir_lowering=False, debug=False, num_devices=NCORES
    )
    f16 = mybir.dt.float16
    bf16 = mybir.dt.bfloat16
    f32 = mybir.dt.float32

    xs = nc.dram_tensor("xs", [128, NB, S, BC], f16, kind="ExternalInput").ap()
    wb = nc.dram_tensor("wb", [128, T * 7 * 128], f16, kind="ExternalInput").ap()
    y = nc.dram_tensor("y", [T, NB, 128, BC], bf16, kind="ExternalOutput").ap()

    KW = 7 * 128

    with tile.TileContext(nc) as tc, ExitStack() as ctx:
        xpool = ctx.enter_context(tc.tile_pool(name="x", bufs=1))
        wpool = ctx.enter_context(tc.tile_pool(name="w", bufs=1))
        ppool = ctx.enter_context(tc.tile_pool(name="ps", bufs=6, space="PSUM"))
        spool = ctx.enter_context(tc.tile_pool(name="sp", bufs=1, space="PSUM"))
        opool = ctx.enter_context(tc.tile_pool(name="o", bufs=6))

        xt = xpool.tile([128, NB * S * BC], f16, tag="xslab")
        xt4 = xt[:].rearrange("p (c s b) -> p c s b", c=NB, s=S)
        wt = wpool.tile([128, T * KW], f16, tag="wslab")
        wsp = wpool.tile([128, 64], f16, tag="wspin")
        psp = spool.tile([128, 64], f32, tag="pspin")
        warm = opool.tile([128, 1], f32, tag="warm")

        # ---- PE warm-up spins (HAM clock gate needs ~3.4us of PE busy).
        nc.vector.memset(wsp[:], 0.0)
        nc.vector.memset(warm[:], 0.0)
        for _ in range(NSPIN):
            nc.tensor.matmul(
                psp[0:64, :], wsp[:, 0:64], wsp[:], start=True, stop=True
            )

        # Every DMA ring is an in-order queue only if we SAY so: the Tile
        # scheduler is free to reorder same-engine instructions, so chain
        # each ring explicitly (a paced DMA scheduled ahead of the critical
        # loads on its ring would otherwise head-of-line deadlock).
        last_on = {}

        def ring_dma(ring, out, in_):
            eng = {"sync": nc.sync, "scalar": nc.scalar, "pool": nc.gpsimd}[ring]
            d = eng.dma_start(out=out, in_=in_)
            if ring in last_on:
                add_dep_helper(d.ins, last_on[ring].ins, False, f"{ring} order")
            last_on[ring] = d
            return d

        # ---- critical-first loads, row-granular: matmul d of tile 0 needs
        # only x row d (131KB) and one wt slice, so the PE starts as soon as
        # row 0 lands (~8.5us) and pipelines with the incoming stream (the
        # cold-PE matmul rate ~427ns matches the per-row transfer cadence).
        ring_dma("sync", xt4[:, 0, 0:1, :], xs[:, 0, 0:1, :])
        ring_dma("scalar", xt4[:, 0, 1:2, :], xs[:, 0, 1:2, :])
        ring_dma("pool", wt[:, 0 : 3 * 128], wb[:, 0 : 3 * 128])
        ring_dma("sync", xt4[:, 0, 2:3, :], xs[:, 0, 2:3, :])
        ring_dma("scalar", xt4[:, 0, 3:4, :], xs[:, 0, 3:4, :])
        ring_dma("sync", xt4[:, 0, 4:5, :], xs[:, 0, 4:5, :])
        ring_dma("scalar", xt4[:, 0, 5:6, :], xs[:, 0, 5:6, :])
        ring_dma("pool", wt[:, 3 * 128 : KW], wb[:, 3 * 128 : KW])
        ring_dma("sync", xt4[:, 0, 6:7, :], xs[:, 0, 6:7, :])
        # near-critical: tile pair 0 c=1 rows + tile 1 weights
        ring_dma("scalar", xt4[:, 1, 0:2, :], xs[:, 1, 0:2, :])
        ring_dma("sync", xt4[:, 1, 2:4, :], xs[:, 1, 2:4, :])
        ring_dma("scalar", xt4[:, 1, 4:6, :], xs[:, 1, 4:6, :])
        ring_dma("sync", xt4[:, 1, 6:7, :], xs[:, 1, 6:7, :])
        ring_dma("pool", wt[:, KW : 2 * KW], wb[:, KW : 2 * KW])
        # sigmoid table warm-up (ACT loads its table during the load phase)
        nc.scalar.activation(warm[:], warm[:], mybir.ActivationFunctionType.Sigmoid)

        # wb tiles 2..15: on the Pool SWDGE ring, gated on matmul progress
        # (tile t releases when pair t-2 starts) so they never compete with
        # nearer-term transfers.
        wb_dmas = {}
        for t in range(2, T):
            wb_dmas[t] = ring_dma(
                "pool", wt[:, t * KW : (t + 1) * KW], wb[:, t * KW : (t + 1) * KW]
            )

        # x row-groups: interleaved into the compute stream below; group k
        # (rows 7+3k..9+3k, both chunks) is emitted at tile pair 3k-1 so the
        # SP ring releases it just-in-time.
        xgroup_at = {max(3 * k - 1, 0): k for k in range(5)}

        first_mm = {}
        for t in range(T):
            if t in xgroup_at:
                k = xgroup_at[t]
                lo, hi = 7 + 3 * k, min(10 + 3 * k, S)
                ring_dma("sync", xt4[:, 0, lo:hi, :], xs[:, 0, lo:hi, :])
                ring_dma("sync", xt4[:, 1, lo:hi, :], xs[:, 1, lo:hi, :])
            for c in range(NB):
                ps = ppool.tile([128, BC], f32, tag="ps")
                for d in range(7):
                    lhs = wt[:, (t * 7 + d) * 128 : (t * 7 + d + 1) * 128]
                    rhs = xt4[:, c, t + d, :]
                    mm = nc.tensor.matmul(
                        ps[:], lhs, rhs, start=(d == 0), stop=(d == 6)
                    )
                    if c == 0 and d == 0:
                        first_mm[t] = mm
                ot = opool.tile([128, BC], bf16, tag="o")
                nc.scalar.activation(
                    ot[:], ps[:], mybir.ActivationFunctionType.Sigmoid
                )
                ring_dma("scalar" if c == 0 else "sync", y[t, c], ot[:])

        # pacing edges for the weight stream
        for t in range(2, T):
            add_dep_helper(
                wb_dmas[t].ins,
                first_mm[t - 2].ins,
                True,
                "pace wb stream behind compute",
            )
    nc.compile()
    return nc


def _build_banded(weights: np.ndarray) -> np.ndarray:
    """G[i, d, js, jd] = weight of edge (src row i+d-3, col js) -> (dest row i, col jd).

    dy = 3 - d (dest = src + dy), dx = jd - js, weight index = w[dy+3, dx+3][src_hw].
    """
    w4 = weights.reshape(7, 7, H, W)
    G = np.zeros((H, 7, W, W), np.float32)
    i = np.arange(H)
    for d in range(7):
        r = i + d - 3
        vi = i[(r >= 0) & (r < H)]
        if len(vi) == 0:
            continue
        for dxi in range(7):
            dx = dxi - 3
            js = np.arange(max(0, -dx), W - max(0, dx))
            G[vi[:, None], d, js[None, :], js[None, :] + dx] = w4[6 - d, dxi][
                (vi + d - 3)[:, None], js[None, :]
            ]
    return G


def kernel(x: np.ndarray, weights: np.ndarray) -> np.ndarray:
    global last_exec_time_ns, last_results
    from concourse.bass_utils import run_bass_kernel_spmd

    x = np.ascontiguousarray(x, dtype=np.float32)
    weights = np.ascontiguousarray(weights, dtype=np.float32)

    if "nc" not in _cache:
        _cache["nc"] = _build_program()
    nc = _cache["nc"]

    x3 = x.reshape(H, W, B)
    xp = np.zeros((H + 6, W, B), np.float16)
    xp[3 : H + 3] = x3.astype(np.float16)
    G = _build_banded(weights).astype(np.float16)  # [H, 7, W(js), W(jd)]

    in_maps = []
    for q in range(NCORES):
        blk = xp[T * q : T * q + S]  # [S, W, B]
        # -> [W(partition), NB, S, BC], contiguous
        xh = np.ascontiguousarray(
            blk.transpose(1, 0, 2).reshape(W, S, NB, BC).transpose(0, 2, 1, 3)
        )
        gq = G[T * q : T * q + T]  # [T, 7, W(js), W(jd)]
        # -> [W(js) partition, T*7*W(jd)], contiguous
        wh = np.ascontiguousarray(
            gq.transpose(2, 0, 1, 3).reshape(W, T * 7 * W)
        )
        in_maps.append({"xs": xh, "wb": wh})

    trace = os.environ.get("KERNEL_TRACE", "0") == "1"
    res = run_bass_kernel_spmd(
        nc, in_maps, core_ids=list(range(NCORES)), trace=trace
    )
    last_exec_time_ns = res.exec_time_ns
    last_results = res
    parts = []
    for r in res.results:
        arr = np.asarray(r["y"])  # [T, NB, 128, BC] bf16
        parts.append(
            arr.transpose(0, 2, 1, 3).reshape(T * W, B).astype(np.float32)
        )
    return np.concatenate(parts, axis=0)


# revision 8
# speedup vs baseline: 1.0774x; 1.0774x over previous
"""Trainium2 Bass kernel for nn_BCNLayer (locally-connected 7x7 lattice layer + sigmoid).

Math: y[i,j,b] = sigmoid( sum_{dy,dx in [-3,3]} w[dy+3,dx+3][(i-dy)*W + (j-dx)]
                          * x[(i-dy)*W + (j-dx), b] )   (zero outside lattice)

Strategy:
  - 8-way shard over lattice rows (H=128 -> 16 dest rows/core, 22 source rows
    with 3-row halos, zero-padded at the edges).
  - For one dest row i and source-row offset d (7 of them), the contribution is
    a banded 128x128 matrix (band +-3 over lattice columns) applied to the
    source row's [128 cols x B batch] slab: nc.tensor.matmul(psum, lhsT=Wband,
    rhs=xrow) accumulated over the 7 source rows.  Banded matrices are prebuilt
    on the host and DMA'd in.
  - PE is the bottleneck (224 matmuls x 512 rows at 2.4GHz ~= 48.4us/core), so
    everything is scheduled around keeping it 100% busy from the earliest start:
      * tile order is t-outer / batch-chunk-inner so each wb tile serves two
        consecutive psum tiles (halves weight-stream bandwidth demand).
      * critical-first loads: tile 0's operands split across all three DMA
        paths (SP HWDGE, ACT HWDGE, Pool SWDGE) with nothing else competing.
      * bulk loads are paced: x row-groups are interleaved into the compute
        stream (engine FIFOs release them just-in-time), wb tiles gated on
        matmul progress via explicit deps, so in-flight DMA never floods the
        HBM path that the next-needed transfer is on.
      * spin matmuls keep the PE busy through the load phase so the HAM clock
        gate is warm (2.4GHz) when real matmuls start.
      * outputs are bf16 (host upcasts; bf16 keeps fp32's exponent range so
        tiny sigmoid outputs stay accurate) on the ACT/SP HWDGE rings.
"""

import os

import numpy as np

H = 128
W = 128
HW = H * W
B = 1024
NCORES = 8
T = H // NCORES  # dest rows per core = 16
S = T + 6        # source rows per core (halo 3 each side) = 22
BC = 512         # batch chunk (psum bank = 512 fp32)
NB = B // BC     # chunks = 2
NSPIN = 95       # PE warm-up matmuls during the load phase

_cache: dict = {}

# filled by the last kernel() call when KERNEL_TRACE=1
last_exec_time_ns = None
last_results = None


def _build_program():
    from contextlib import ExitStack

    import concourse.bacc as bacc
    import concourse.mybir as mybir
    import concourse.tile as tile
    from concourse.tile_rust import add_dep_helper

    nc = bacc.Bacc(
        "TRN2", target_bir_lowering=False, debug=False, num_devices=NCORES
    )
    f16 = mybir.dt.float16
    bf16 = mybir.dt.bfloat16
    f32 = mybir.dt.float32

    xs = nc.dram_tensor("xs", [128, NB, S, BC], f16, kind="ExternalInput").ap()
    wb = nc.dram_tensor("wb", [128, T * 7 * 128], f16, kind="ExternalInput").ap()
    y = nc.dram_tensor("y", [T, NB, 128, BC], bf16, kind="ExternalOutput").ap()

    KW = 7 * 128

    with tile.TileContext(nc) as tc, ExitStack() as ctx:
        xpool = ctx.enter_context(tc.tile_pool(name="x", bufs=1))
        wpool = ctx.enter_context(tc.tile_pool(name="w", bufs=1))
        ppool = ctx.enter_context(tc.tile_pool(name="ps", bufs=6, space="PSUM"))
        spool = ctx.enter_context(tc.tile_pool(name="sp", bufs=1, space="PSUM"))
        opool = ctx.enter_context(tc.tile_pool(name="o", bufs=6))

        xt = xpool.tile([128, NB * S * BC], f16, tag="xslab")
        xt4 = xt[:].rearrange("p (c s b) -> p c s b", c=NB, s=S)
        wt = wpool.tile([128, T * KW], f16, tag="wslab")
        wsp = wpool.tile([128, 64], f16, tag="wspin")
        psp = spool.tile([128, 64], f32, tag="pspin")
        warm = opool.tile([128, 1], f32, tag="warm")

        # ---- PE warm-up spins (HAM clock gate needs ~3.4us of PE busy).
        nc.vector.memset(wsp[:], 0.0)
        nc.vector.memset(warm[:], 0.0)
        for _ in range(NSPIN):
            nc.tensor.matmul(
                psp[0:64, :], wsp[:, 0:64], wsp[:], start=True, stop=True
            )

        # Every DMA ring is an in-order queue only if we SAY so: the Tile
        # scheduler is free to reorder same-engine instructions, so chain
        # each ring explicitly (a paced DMA scheduled ahead of the critical
        # loads on its ring would otherwise head-of-line deadlock).
        last_on = {}

        def ring_dma(ring, out, in_):
            eng = {"sync": nc.sync, "scalar": nc.scalar, "pool": nc.gpsimd}[ring]
            d = eng.dma_start(out=out, in_=in_)
            if ring in last_on:
                add_dep_helper(d.ins, last_on[ring].ins, False, f"{ring} order")
            last_on[ring] = d
            return d

        # ---- preload: tile pair 0's operands (both chunks of rows 0-6 +
        # wt0/wt1), 2-row granular, byte-balanced across the three DMA paths
        # so everything lands ~12us while spin matmuls keep the clock warm.
        ring_dma("sync", xt4[:, 0, 0:2, :], xs[:, 0, 0:2, :])
        ring_dma("scalar", xt4[:, 0, 2:4, :], xs[:, 0, 2:4, :])
        ring_dma("pool", wt[:, 0:KW], wb[:, 0:KW])
        ring_dma("sync", xt4[:, 0, 4:6, :], xs[:, 0, 4:6, :])
        ring_dma("scalar", xt4[:, 0, 6:7, :], xs[:, 0, 6:7, :])
        ring_dma("sync", xt4[:, 1, 2:4, :], xs[:, 1, 2:4, :])
        ring_dma("scalar", xt4[:, 1, 0:2, :], xs[:, 1, 0:2, :])
        ring_dma("pool", wt[:, KW : 2 * KW], wb[:, KW : 2 * KW])
        ring_dma("sync", xt4[:, 1, 6:7, :], xs[:, 1, 6:7, :])
        ring_dma("scalar", xt4[:, 1, 4:6, :], xs[:, 1, 4:6, :])
        # sigmoid table warm-up (ACT loads its table during the load phase)
        nc.scalar.activation(warm[:], warm[:], mybir.ActivationFunctionType.Sigmoid)

        # wb tiles 2..15: on the Pool SWDGE ring, gated on matmul progress
        # (tile t releases when pair t-2 starts) so they never compete with
        # nearer-term transfers.
        wb_dmas = {}
        for t in range(2, T):
            wb_dmas[t] = ring_dma(
                "pool", wt[:, t * KW : (t + 1) * KW], wb[:, t * KW : (t + 1) * KW]
            )

        # x row-groups: interleaved into the compute stream below; group k
        # (rows 7+3k..9+3k, both chunks) is emitted at tile pair 3k-1 so the
        # SP ring releases it just-in-time (group 0 right at pair 0).
        xgroup_at = {0: 0, 2: 1, 5: 2, 8: 3, 11: 4}

        first_mm = {}
        for t in range(T):
            if t in xgroup_at:
                k = xgroup_at[t]
                lo, hi = 7 + 3 * k, min(10 + 3 * k, S)
                ring_dma("sync", xt4[:, 0, lo:hi, :], xs[:, 0, lo:hi, :])
                ring_dma("sync", xt4[:, 1, lo:hi, :], xs[:, 1, lo:hi, :])
            for c in range(NB):
                ps = ppool.tile([128, BC], f32, tag="ps")
                for d in range(7):
                    lhs = wt[:, (t * 7 + d) * 128 : (t * 7 + d + 1) * 128]
                    rhs = xt4[:, c, t + d, :]
                    mm = nc.tensor.matmul(
                        ps[:], lhs, rhs, start=(d == 0), stop=(d == 6)
                    )
                    if c == 0 and d == 0:
                        first_mm[t] = mm
                ot = opool.tile([128, BC], bf16, tag="o")
                nc.scalar.activation(
                    ot[:], ps[:], mybir.ActivationFunctionType.Sigmoid
                )
                ring_dma("scalar" if c == 0 else "sync", y[t, c], ot[:])

        # pacing edges for the weight stream
        for t in range(2, T):
            add_dep_helper(
                wb_dmas[t].ins,
                first_mm[max(t - 3, 0)].ins,
                True,
                "pace wb stream behind compute",
            )
    nc.compile()
    return nc


def _build_banded(weights: np.ndarray) -> np.ndarray:
    """G[i, d, js, jd] = weight of edge (src row i+d-3, col js) -> (dest row i, col jd).

    dy = 3 - d (dest = src + dy), dx = jd - js, weight index = w[dy+3, dx+3][src_hw].
    """
    w4 = weights.reshape(7, 7, H, W)
    G = np.zeros((H, 7, W, W), np.float32)
    i = np.arange(H)
    for d in range(7):
        r = i + d - 3
        vi = i[(r >= 0) & (r < H)]
        if len(vi) == 0:
            continue
        for dxi in range(7):
            dx = dxi - 3
            js = np.arange(max(0, -dx), W - max(0, dx))
            G[vi[:, None], d, js[None, :], js[None, :] + dx] = w4[6 - d, dxi][
                (vi + d - 3)[:, None], js[None, :]
            ]
    return G


def kernel(x: np.ndarray, weights: np.ndarray) -> np.ndarray:
    global last_exec_time_ns, last_results
    from concourse.bass_utils import run_bass_kernel_spmd

    x = np.ascontiguousarray(x, dtype=np.float32)
    weights = np.ascontiguousarray(weights, dtype=np.float32)

    if "nc" not in _cache:
        _cache["nc"] = _build_program()
    nc = _cache["nc"]

    x3 = x.reshape(H, W, B)
    xp = np.zeros((H + 6, W, B), np.float16)
    xp[3 : H + 3] = x3.astype(np.float16)
    G = _build_banded(weights).astype(np.float16)  # [H, 7, W(js), W(jd)]

    in_maps = []
    for q in range(NCORES):
        blk = xp[T * q : T * q + S]  # [S, W, B]
        # -> [W(partition), NB, S, BC], contiguous
        xh = np.ascontiguousarray(
            blk.transpose(1, 0, 2).reshape(W, S, NB, BC).transpose(0, 2, 1, 3)
        )
        gq = G[T * q : T * q + T]  # [T, 7, W(js), W(jd)]
        # -> [W(js) partition, T*7*W(jd)], contiguous
        wh = np.ascontiguousarray(
            gq.transpose(2, 0, 1, 3).reshape(W, T * 7 * W)
        )
        in_maps.append({"xs": xh, "wb": wh})

    trace = os.environ.get("KERNEL_TRACE", "0") == "1"
    res = run_bass_kernel_spmd(
        nc, in_maps, core_ids=list(range(NCORES)), trace=trace
    )
    last_exec_time_ns = res.exec_time_ns
    last_results = res
    parts = []
    for r in res.results:
        arr = np.asarray(r["y"])  # [T, NB, 128, BC] bf16
        parts.append(
            arr.transpose(0, 2, 1, 3).reshape(T * W, B).astype(np.float32)
        )
    return np.concatenate(parts, axis=0)
